# revision 38
# baseline (speedup 1.0000x reference)
"""GQA multi-head attention (B=2, S=2048, H=2048, 16 Q heads / 4 KV heads, RoPE,
causal) on 8 Trainium2 NeuronCores.

Sharding: tensor-parallel over GQA groups (4 groups, each 4 Q heads + 1 KV head)
x data-parallel over batch (2). Core c handles batch b = c // 4, group g = c % 4.
Column-parallel q/k/v projections, row-parallel o_proj; the 4 partial o_proj
outputs per batch are summed on the host.

Per-core kernel (all matmuls bf16 with fp32 PSUM accumulation):
  phase A: K^T/V^T projections chunk-streamed together (8 matmuls per hidden
           chunk > chunk DMA time, so the PE never starves while x loads).
           Host-reshaped weights keep every DMA row >=4KB contiguous. K/V
           results copied PSUM->SBUF bf16 immediately so the PSUM banks free
           without waiting for the cos/sin tables; RoPE runs from SBUF in
           bf16 (DVE 2x mode), half-swap via two ScalarE half-partition
           copies.
  phase B: V transposed to natural layout on PE (fills the x-load tail), then
           Q^T projections + RoPE.
  phase C: flash-style attention in S^T (keys x queries) layout, processed in
           pairs of 128-key blocks: S^T per block on PE, ONE exp per pair on
           ScalarE over the 2-bank PSUM pair (halves ACT overhead), causal
           mask via an additive 128x128 matmul on diagonal blocks, key-axis
           row sums via a ones-vector matmul, O^T = V^T P^T in PSUM,
           normalized by 1/rowsum. o_proj yp tiles share the osum PSUM ring
           (their OT dependency coincides with the ring WAR, so QK/exp of the
           next tile keep flowing); y staged in SBUF, one 1MB DMA per token
           block on the GpSimd queue to keep the Sync queue fast for the
           small rowsum-reshape DMAs.
"""

import sys

for _p in ("/root/.axon_site", "/root/.axon_site/_ro/trn_rl_repo",
           "/root/.axon_site/_ro/pypackages", "/opt/trn_rl_repo"):
    if _p not in sys.path:
        sys.path.append(_p)

import numpy as np
import ml_dtypes

import concourse.bass as bass
import concourse.tile as tile
import concourse.mybir as mybir
from concourse import bacc
from concourse.bass import ts
from concourse.bass_utils import run_bass_kernel_spmd
from concourse.masks import make_identity, make_upper_triangular
from contextlib import ExitStack

BF16 = ml_dtypes.bfloat16
P = 128
S = 2048
H = 2048
NH = 4          # Q heads per core
DQ = NH * P     # 512
NCH = H // P    # 16 hidden chunks
NKB = S // P    # 16 key blocks
QTS = 512       # query tile
SCALE = 1.0 / float(np.sqrt(128.0))


def build_nc():
    f32 = mybir.dt.float32
    bf16 = mybir.dt.bfloat16
    nc = bacc.Bacc("TRN2", target_bir_lowering=False, debug=False)

    xT = nc.dram_tensor("xT", (H, S), bf16, kind="ExternalInput").ap()
    # wq stored as [p, head-pair, chunk, 256] so each half-load is contiguous
    wqA = nc.dram_tensor("wqA", (P, 2, NCH, DQ // 2), bf16,
                         kind="ExternalInput").ap()
    wkA = nc.dram_tensor("wkA", (P, NCH, P), bf16, kind="ExternalInput").ap()
    wvA = nc.dram_tensor("wvA", (P, NCH, P), bf16, kind="ExternalInput").ap()
    woA = nc.dram_tensor("woA", (P, NH, H), bf16, kind="ExternalInput").ap()
    cosT = nc.dram_tensor("cosT", (P, S), bf16, kind="ExternalInput").ap()
    srT = nc.dram_tensor("sinrotT", (P, S), bf16, kind="ExternalInput").ap()
    y = nc.dram_tensor("y", (S, H), f32, kind="ExternalOutput").ap()

    Exp = mybir.ActivationFunctionType.Exp

    with ExitStack() as ctx:
        tc = ctx.enter_context(tile.TileContext(nc))
        singles = ctx.enter_context(tc.tile_pool(name="singles", bufs=1))

        xT_sb = singles.tile([P, NCH, S], bf16)
        wqT_sb = singles.tile([P, 2, NCH, DQ // 2], bf16)
        wkT_sb = singles.tile([P, NCH, P], bf16)
        wvT_sb = singles.tile([P, NCH, P], bf16)
        cos_sb = singles.tile([P, S], bf16)
        sr_sb = singles.tile([P, S], bf16)
        woT_sb = singles.tile([P, NH, H], bf16)
        xTr = xT.rearrange("(c p) s -> p c s", p=P)
        # The 16 DMA engines round-robin over all queued transfers, so issuing
        # everything up front lets late-needed weights steal HBM bandwidth
        # from the x stream that gates every projection. Stagger the issues
        # with scheduler-time waits so the load order is approximately
        # sequential: x first, then wq (needed when Q proj starts), then the
        # RoPE tables, then wo (needed only by o_proj ~120us in).
        nc.sync.dma_start(wkT_sb[:, :, :], wkA)
        nc.sync.dma_start(wvT_sb[:, :, :], wvA)
        for c in range(4):
            nc.sync.dma_start(xT_sb[:, c, :], xTr[:, c, :])
        for gi, c in enumerate(range(4, 16, 4)):
            with tc.tile_wait_until(0.012 + 0.008 * gi):
                nc.sync.dma_start(xT_sb[:, c:c + 4, :], xTr[:, c:c + 4, :])
        with tc.tile_wait_until(0.038):
            nc.sync.dma_start(wqT_sb[:, 0, :, :], wqA[:, 0, :, :])
        with tc.tile_wait_until(0.042):
            nc.sync.dma_start(wqT_sb[:, 1, :, :], wqA[:, 1, :, :])
        with tc.tile_wait_until(0.046):
            nc.sync.dma_start(cos_sb, cosT)
            nc.sync.dma_start(sr_sb, srT)
        with tc.tile_wait_until(0.050):
            nc.sync.dma_start(woT_sb[:, :, :], woA)

        # Additive causal mask, applied on the PE: a matmul of masknegT.T @ I
        # accumulated into the scores PSUM adds -1e9 where key > query.
        masknegT = singles.tile([P, P], bf16)
        make_upper_triangular(nc, masknegT[:], val=-1e9, diag=False)
        ident = singles.tile([P, P], bf16)
        make_identity(nc, ident[:])
        ones = singles.tile([P, 1], bf16)
        nc.vector.memset(ones[:], 1.0)
        zeros = singles.tile([P, P], bf16)
        nc.vector.memset(zeros[:], 0.0)

        QT_sb = singles.tile([P, NH, S], bf16)
        KT_sb = singles.tile([P, S], bf16)
        VT_sb = singles.tile([P, S], bf16)
        Vn_sb = singles.tile([P, NKB, P], bf16)
        OT_sb = singles.tile([P, NH, S], bf16)

        def rope(rp, qf, out_region, tok):
            # qf: pre-rope values, bf16 in SBUF. Half-swap via two ScalarE
            # half-partition copies (ACT is idle in phases A/B); multiplies in
            # bf16 hit the DVE 2x mode. Sign of sin pre-folded host-side.
            sw = rp.tile([P, QTS], bf16, tag="sw")
            nc.scalar.copy(sw[0:64, :], qf[64:128, :])
            nc.scalar.copy(sw[64:128, :], qf[0:64, :])
            t1 = rp.tile([P, QTS], bf16, tag="t1")
            nc.vector.tensor_mul(t1[:, :], qf[:, :], cos_sb[:, ts(tok, QTS)])
            t2 = rp.tile([P, QTS], bf16, tag="t2")
            nc.vector.tensor_mul(t2[:, :], sw[:, :], sr_sb[:, ts(tok, QTS)])
            nc.vector.tensor_add(out_region, t1[:, :], t2[:, :])

        # ---------------- phase A: K/V projections chunk-streamed -----------
        with tc.tile_pool(name="pjA", bufs=1, space="PSUM") as pjA, \
             tc.tile_pool(name="ropeA", bufs=2) as rpA:
            kps = [pjA.tile([P, QTS], f32, tag=f"k{t}", name=f"kps{t}")
                   for t in range(4)]
            vps = [pjA.tile([P, QTS], f32, tag=f"v{t}", name=f"vps{t}")
                   for t in range(4)]
            # PE warmup during the initial DMA window (discarded by the
            # start=True of the real chain); keeps HAM at full clock.
            for _ in range(40):
                nc.tensor.matmul(kps[0][:, 0:P], ident[:], ident[:],
                                 start=True, stop=True)
            for c in range(NCH):
                for t in range(4):
                    nc.tensor.matmul(kps[t][:, :], wkT_sb[:, c, :],
                                     xT_sb[:, c, ts(t, QTS)],
                                     start=(c == 0), stop=(c == NCH - 1))
                for t in range(4):
                    nc.tensor.matmul(vps[t][:, :], wvT_sb[:, c, :],
                                     xT_sb[:, c, ts(t, QTS)],
                                     start=(c == 0), stop=(c == NCH - 1))
                if 0 < c < 12:
                    # accumulate-zero matmuls fill the per-chunk DMA-wait
                    # micro-gaps so the HAM activity window stays busy and the
                    # PE holds its full 2.4GHz clock through the x stream.
                    for _ in range(3):
                        nc.tensor.matmul(kps[0][:, 0:P], ident[:], zeros[:],
                                         start=False, stop=False,
                                         skip_group_check=True)
            # evacuate PSUM immediately (bf16): frees the 8 banks for phase B
            # without waiting for the cos/sin tables needed by RoPE.
            kf_sb = singles.tile([P, 4, QTS], bf16)
            for t in range(4):
                nc.vector.tensor_copy(kf_sb[:, t, :], kps[t][:, :])
                nc.vector.tensor_copy(VT_sb[:, ts(t, QTS)], vps[t][:, :])

        with tc.tile_pool(name="ropeA2", bufs=2) as rpA2:
            for t in range(4):
                rope(rpA2, kf_sb[:, t, :], KT_sb[:, ts(t, QTS)], t)

        # ---------------- phase B: V transpose, Q projections + RoPE --------
        with tc.tile_pool(name="pjB", bufs=4, space="PSUM") as pjB, \
             tc.tile_pool(name="ropeB", bufs=3) as rpB, \
             tc.tile_pool(name="vtp", bufs=2, space="PSUM") as vtp:
            for b in range(NKB):
                tp = vtp.tile([P, P], bf16, tag="vt")
                nc.tensor.transpose(tp[:, :], VT_sb[:, ts(b, P)], ident[:])
                nc.vector.tensor_copy(Vn_sb[:, b, :], tp[:, :])
            for h in range(NH):
                # two q tiles' accumulation chains interleaved so consecutive
                # matmuls target different PSUM banks (drain overlaps fill).
                for t0 in range(0, S // QTS, 2):
                    pss = [pjB.tile([P, QTS], f32, tag="q", name=f"qp{t0 + u}")
                           for u in range(2)]
                    for c in range(NCH):
                        for u in range(2):
                            nc.tensor.matmul(
                                pss[u][:, :],
                                wqT_sb[:, h // 2, c, ts(h % 2, P)],
                                xT_sb[:, c, ts(t0 + u, QTS)],
                                start=(c == 0), stop=(c == NCH - 1))
                    for u in range(2):
                        qf = rpB.tile([P, QTS], bf16, tag="qf")
                        nc.vector.tensor_copy(qf[:, :], pss[u][:, :])
                        rope(rpB, qf, QT_sb[:, h, ts(t0 + u, QTS)], t0 + u)

        # ---------------- phase C: attention + o_proj interleaved -----------
        # PSUM budget (8 banks): sp pair ring 2x[128,2,512] = 4, osum 2,
        # yp 1, rs 1. o_proj yp groups for token tile t are drained one group
        # at a time into the attention pair-loop of tile t-1 (from its second
        # head on), filling the PE slack of the ACT-bound attention stretch
        # instead of running as a separate burst while ScalarE idles.
        with tc.tile_pool(name="spp", bufs=2, space="PSUM") as spp, \
             tc.tile_pool(name="opp", bufs=2, space="PSUM") as opp, \
             tc.tile_pool(name="rsp", bufs=1, space="PSUM") as rsp, \
             tc.tile_pool(name="ypp", bufs=1, space="PSUM") as ypp, \
             tc.tile_pool(name="ptp", bufs=4) as ptp, \
             tc.tile_pool(name="yop", bufs=2) as yop, \
             tc.tile_pool(name="nrm", bufs=3) as nrm:
            pending = []    # (tb, ho) o_proj groups ready to emit
            yo_cur = [None]

            def emit_oproj_group():
                tb, ho = pending.pop(0)
                if ho == 0:
                    yo_cur[0] = yop.tile([P, H], f32, tag="yo", name="yo")
                yo = yo_cur[0]
                yp = ypp.tile([P, QTS], f32, tag="yp", name="yp")
                for h in range(NH):
                    nc.tensor.matmul(yp[:, :], OT_sb[:, h, ts(tb, P)],
                                     woT_sb[:, h, ts(ho, QTS)],
                                     start=(h == 0), stop=(h == NH - 1))
                nc.vector.tensor_copy(yo[:, ts(ho, QTS)], yp[:, :])
                if ho == 1:
                    nc.sync.dma_start(y[ts(tb, P), 0:2 * QTS], yo[:, 0:2 * QTS])
                elif ho == NH - 1:
                    nc.sync.dma_start(y[ts(tb, P), 2 * QTS:H],
                                      yo[:, 2 * QTS:H])

            for t in range(S // QTS):
                for h in range(NH):
                    qs = QTS * t
                    nj = 4 * t + 4          # key blocks for this q tile
                    npair = nj // 2
                    osum = opp.tile([P, QTS], f32, tag="osum")
                    rs = rsp.tile([1, QTS], f32, tag="rs")
                    # pipelined over block pairs with lag 2: the QK of pairs
                    # i+1 AND i+2 sit ahead of the exp(i)-gated rowsum/PV in
                    # the PE queue, so consecutive exps run back-to-back on
                    # ScalarE. The pair presum runs at lag 1 so the rowsum
                    # matmul at lag 2 never waits on the DVE.
                    LAG = 2
                    pts, psums, cos_ = [None] * npair, [None] * npair, [0] * nj
                    for ip in range(npair + LAG):
                        if ip < npair:
                            sp2 = spp.tile([P, 2, QTS], f32, tag="sp")
                            for b in range(2):
                                j = 2 * ip + b
                                co = max(0, P * j - qs)
                                cos_[j] = co
                                diag = j >= 4 * t
                                nc.tensor.matmul(
                                    sp2[:, b, co:QTS], KT_sb[:, ts(j, P)],
                                    QT_sb[:, h, qs + co:qs + QTS],
                                    start=True, stop=not diag)
                                if diag:
                                    nc.tensor.matmul(
                                        sp2[:, b, co:co + P], masknegT[:],
                                        ident[:], start=False, stop=True)
                            pt2 = ptp.tile([P, 2, QTS], bf16, tag="pt")
                            comin = cos_[2 * ip]
                            nc.scalar.activation(pt2[:, :, comin:QTS],
                                                 sp2[:, :, comin:QTS],
                                                 Exp, scale=SCALE)
                            pts[ip] = pt2
                        if 1 <= ip < npair + 1:
                            iq = ip - 1
                            pt2 = pts[iq]
                            c0, c1 = cos_[2 * iq], cos_[2 * iq + 1]
                            if c1 > c0:
                                # diagonal pair: zero the strip of block j1
                                # that exp filled from stale PSUM so the
                                # pair presum is valid over [c0:QTS).
                                nc.vector.memset(pt2[:, 1, c0:c1], 0.0)
                            # pair presum on DVE (bf16 2x mode) halves the
                            # rowsum matmul stream on the PE.
                            psum2 = ptp.tile([P, QTS], bf16, tag="ptsum")
                            nc.vector.tensor_add(psum2[:, c0:QTS],
                                                 pt2[:, 0, c0:QTS],
                                                 pt2[:, 1, c0:QTS])
                            psums[iq] = psum2
                        if ip >= LAG:
                            iq = ip - LAG
                            pt2 = pts[iq]
                            c0 = cos_[2 * iq]
                            nc.tensor.matmul(
                                rs[0:1, c0:QTS], ones[:],
                                psums[iq][:, c0:QTS],
                                start=(iq == 0), stop=(iq == npair - 1))
                            for b in range(2):
                                j = 2 * iq + b
                                co = cos_[j]
                                nc.tensor.matmul(
                                    osum[:, co:QTS], Vn_sb[:, j, :],
                                    pt2[:, b, co:QTS],
                                    start=(j == 0), stop=(j == nj - 1))
                            # drain pending o_proj groups into the PE slack
                            # of the ACT-bound attention stretch (only once
                            # this tile's first head is done, so the previous
                            # tile's last recip chain has completed and the
                            # group's OT reads don't stall the in-order PE).
                            if pending and h >= 1 and \
                                    (ip % 2 == 0 or len(pending) > 8):
                                emit_oproj_group()
                    # 1/rowsum: DMA-reshape the 512 sums over 128 partitions
                    # (a 1-partition DVE reciprocal costs 3.3us), reciprocal,
                    # reshape back, broadcast across partitions on GpSimd.
                    rsc = nrm.tile([1, QTS], f32, tag="rsc")
                    nc.vector.tensor_copy(rsc[:, :], rs[0:1, :])
                    rv = nrm.tile([P, 4], f32, tag="rv")
                    nc.sync.dma_start(rv[:, :], rsc[:, :])
                    rvr = nrm.tile([P, 4], f32, tag="rvr")
                    nc.vector.reciprocal(rvr[:, :], rv[:, :])
                    rrow = nrm.tile([1, QTS], f32, tag="rrow")
                    nc.sync.dma_start(rrow[:, :], rvr[:, :])
                    recipB = nrm.tile([P, QTS], f32, tag="recipB")
                    nc.gpsimd.partition_broadcast(recipB[:, :], rrow[:, :])
                    nc.vector.tensor_mul(OT_sb[:, h, qs:qs + QTS], osum[:, :],
                                         recipB[:, :])
                # queue this tile's o_proj groups; they drain into the next
                # tile's attention pair loop.
                for tb in range(4 * t, 4 * t + 4):
                    for ho in range(H // QTS):
                        pending.append((tb, ho))
            while pending:
                emit_oproj_group()

    nc.compile()
    return nc


_NC_CACHE = None


def _get_nc():
    global _NC_CACHE
    if _NC_CACHE is None:
        _NC_CACHE = build_nc()
    return _NC_CACHE


def make_in_maps(hidden_states, position_ids, wq, wk, wv, wo):
    """Host-side sharding: 8 cores = (batch b = core//4) x (GQA group g = core%4).

    Weights are reshaped so that SBUF destination partition p is the leading
    axis with contiguous rows: A[p, c, m] = W^T[c*128 + p, m].
    """
    def resh(wT, nch):  # (nch*128, m) -> (128, nch, m) contiguous
        m = wT.shape[1]
        return np.ascontiguousarray(
            wT.reshape(nch, P, m).transpose(1, 0, 2)).astype(BF16)

    def resh_q(wT):  # (2048, 512) -> (128, 2, 16, 256) contiguous
        a = wT.reshape(NCH, P, 2, DQ // 2).transpose(1, 2, 0, 3)
        return np.ascontiguousarray(a).astype(BF16)

    in_maps = []
    xTs, coss, srs = {}, {}, {}
    for b in range(2):
        xTs[b] = np.ascontiguousarray(hidden_states[b].T).astype(BF16)
        inv = 1.0 / (10000.0 ** (np.arange(0, P, 2, dtype=np.float64) / P))
        invd = np.concatenate([inv, inv]).astype(np.float64)
        fr = invd[:, None] * position_ids[b].astype(np.float64)[None, :]
        coss[b] = np.cos(fr).astype(BF16)
        sr = np.sin(fr)
        sr[:64] *= -1.0
        srs[b] = sr.astype(BF16)
    shards = {}
    for g in range(4):
        shards[g] = dict(
            wqA=resh_q(wq[DQ * g:DQ * (g + 1)].T),
            wkA=resh(wk[P * g:P * (g + 1)].T, NCH),
            wvA=resh(wv[P * g:P * (g + 1)].T, NCH),
            woA=resh(wo[:, DQ * g:DQ * (g + 1)].T, NH),
        )
    for core in range(8):
        b, g = core // 4, core % 4
        in_maps.append(dict(xT=xTs[b], cosT=coss[b], sinrotT=srs[b], **shards[g]))
    return in_maps


def kernel(hidden_states, position_ids, wq, wk, wv, wo, **run_kwargs):
    nc = _get_nc()
    in_maps = make_in_maps(np.asarray(hidden_states), np.asarray(position_ids),
                           np.asarray(wq), np.asarray(wk), np.asarray(wv),
                           np.asarray(wo))
    res = run_bass_kernel_spmd(nc, in_maps, core_ids=list(range(8)), **run_kwargs)
    out = np.zeros((2, S, H), np.float32)
    for core in range(8):
        out[core // 4] += res.results[core]["y"]
    if run_kwargs:
        kernel.last_results = res
    return out
